# revision 19
# baseline (speedup 1.0000x reference)
"""Trainium2 Bass kernel for FovConv2dCont (per-pixel foveated Gaussian blur + 5x5 conv).

kernel(**inputs): takes FULL inputs
  input_data f32 (8,3,224,224), foa_xy int (8,2), weight f32 (64,3,5,5)
returns f32 (8,64,224,224). Batch is data-parallel across 8 NeuronCores (1 sample/core).

Math (exact identities; bf16 storage on the heavy elementwise chain):
  gaussian tap exp(-(i^2+j^2)/(2 s^2)) = u^(i^2) * u^(j^2),  u = exp(-1/(2 s^2))
  normalizer sum over 7x7 taps = (1 + 2u + 2u^4 + 2u^9)^2
  numerator = sum_e u^e S_e with S_e the sum of taps at squared radius e
  (terms e=13,18 dropped: bounded by ~2e-3 relative, below bf16 noise)
  m = numerator / norm ; y = conv5x5(m, w) with K=90 im2col whose partitions are
  (ci, dy' in 0..5, dx) and lhsT [90,128] = [w | w row-shifted], so each matmul
  column yields TWO output rows (even on psum partitions 0-63, odd on 64-127).
  im2col built in two hops (mpad -> imrow[18] -> icf[90]) so every DMA has a
  large outer AP dim (descriptors spread round-robin over the 16 DMA engines
  by outer-dim index). Output staged bf16, 2 contiguous DMAs per 28-row strip.
"""

import os
import sys

sys.path.insert(0, "/opt/trn_rl_repo")

import numpy as np
import ml_dtypes

def _ensure_ntff_hook():
    """Register the NTFF profile hook if the image's antenv lacks axon_hooks
    (needed only for trace=True timing runs; harmless otherwise)."""
    try:
        import antenv.axon_hooks  # noqa: F401
        return
    except ImportError:
        pass
    try:
        import types
        import antenv
        import importlib.util as ilu

        spec = ilu.spec_from_file_location(
            "trn_agent_boot.trn_boot", "/root/.axon_site/trn_agent_boot/trn_boot.py"
        )
        mod = types.ModuleType("antenv.axon_hooks")
        _hook_holder = {"hook": None}

        def set_axon_ntff_profile_hook(h):
            _hook_holder["hook"] = h

        def get_axon_ntff_profile_hook():
            return _hook_holder["hook"]

        mod.set_axon_ntff_profile_hook = set_axon_ntff_profile_hook
        mod.get_axon_ntff_profile_hook = get_axon_ntff_profile_hook
        sys.modules["antenv.axon_hooks"] = mod
        antenv.axon_hooks = mod

        boot = ilu.module_from_spec(spec)
        spec.loader.exec_module(boot)
        hook = boot._ntff_profile_via_ctypes("/opt/axon/libaxon_pjrt.so")
        set_axon_ntff_profile_hook(hook)
    except Exception:
        pass


_ensure_ntff_hook()

import concourse.bass as bass
import concourse.bacc as bacc_mod
import concourse.mybir as mybir
from concourse.bass_utils import run_bass_kernel_spmd
from concourse.tile import TileContext
from concourse.alu_op_type import AluOpType

F32 = mybir.dt.float32
BF16 = mybir.dt.bfloat16
AF = mybir.ActivationFunctionType

H = W = 224
C = 3
OC = 64
KG = 7
PG = KG // 2            # 3
KC = 5
PC = KC // 2            # 2
WP = W + 2 * PG         # 230
SR = 8                  # strip rows per partition of xs
NP = H // 2             # 112 partitions
MW = W + 2 * PC         # 228
DNORM = float(np.sqrt(H * H + W * W))

NDY = KC + 1            # 6 dy' values (row-pair trick)
K90 = C * NDY * KC      # 90 contraction size
RS = 28                 # output rows per conv strip
NSTRIP = H // RS        # 8
FSIF = RS * MW          # 6384 icf free size
RUNF = (RS - 2) * MW + W  # 6152 elems DMAed per (ci,dy',dx)
FSST = RS * W // 2      # 3136 stage free size (per parity)
NCH = RS // 4           # 7 matmul chunks per strip (4 rows each)
NMM = 2 * W             # 448 matmul N (2 even/odd row pairs x 224)

EXPS_PAIR = {1: (0, 1), 4: (0, 2), 5: (1, 2), 9: (0, 3), 10: (1, 3)}
EXPS_DIAG = {2: 1, 8: 2}
ALL_E = sorted(set(EXPS_PAIR) | set(EXPS_DIAG))

LAST_RESULTS = None
_CACHED = None


def _v(ap_src, offset_elems, dims):
    """Raw strided (possibly overlapping/broadcast) view of a flat AP.
    dims = [(step, count), ...]; for SBUF/PSUM the first dim(s) must cover
    partitions (step in flat units = partition_step * free_size)."""
    fv = ap_src.flatten()
    v = fv.copy()
    v.offset = fv.offset + offset_elems
    v.ap = mybir.VecI64Pair([list(d) for d in dims])
    return v


def _build_nc():
    nc = bacc_mod.Bacc()

    xp = nc.declare_dram_parameter("xp", [C, WP, WP], BF16, isOutput=False)
    av = nc.declare_dram_parameter("av", [H], F32, isOutput=False)
    bv = nc.declare_dram_parameter("bv", [H], F32, isOutput=False)
    wc = nc.declare_dram_parameter("wc", [K90, 2 * OC], BF16, isOutput=False)
    zv = nc.declare_dram_parameter("zv", [2 * C * MW], BF16, isOutput=False)
    out = nc.declare_dram_parameter("out", [OC, H, W], BF16, isOutput=True)

    with TileContext(nc) as tc:
        with (
            tc.tile_pool(name="pers", bufs=1) as pers,
            tc.tile_pool(name="psum", bufs=8, space="PSUM") as psum_pool,
            tc.tile_pool(name="imr", bufs=2) as imr_pool,
            tc.tile_pool(name="icf", bufs=4) as icf_pool,
            tc.tile_pool(name="stg", bufs=2) as stg_pool,
            tc.tile_pool(name="dram", bufs=1, space="DRAM") as dram_pool,
        ):
            mpad = dram_pool.tile([C, MW, MW], BF16)
            XFS = C * SR * WP                       # xs free size 5520
            xs = pers.tile([NP, XFS], BF16)
            CFS = 2 * W                             # coeff free size 448
            at = pers.tile([NP, 2], F32)
            bvf = pers.tile([NP, W], F32)
            d2 = pers.tile([NP, CFS], F32)
            dist = pers.tile([NP, CFS], F32)
            sig = pers.tile([NP, CFS], F32)
            sqv = pers.tile([NP, CFS], F32)
            isg = pers.tile([NP, CFS], F32)
            u1f = pers.tile([NP, CFS], F32)
            u4f = pers.tile([NP, CFS], F32)
            u9f = pers.tile([NP, CFS], F32)
            t1 = pers.tile([NP, CFS], F32)
            t2 = pers.tile([NP, CFS], F32)
            sfield = pers.tile([NP, CFS], F32)
            rsf = pers.tile([NP, CFS], F32)
            rb = pers.tile([NP, CFS], BF16)
            ub = {e: pers.tile([NP, CFS], BF16, name=f"ub{e}") for e in ALL_E}
            RFS = C * 2 * WP                        # rowpair free size 1380
            rp = {a: pers.tile([NP, RFS], BF16, name=f"rp{a}") for a in (1, 2, 3)}
            PFS = C * 2 * W                         # P tile free size 1344
            ptiles = {}
            for e, (a, b) in EXPS_PAIR.items():
                ptiles[(a, b)] = pers.tile([NP, PFS], BF16, name=f"p{a}{b}")
                if a != 0:
                    ptiles[(b, a)] = pers.tile([NP, PFS], BF16, name=f"p{b}{a}")
            for e, a in EXPS_DIAG.items():
                ptiles[(a, a)] = pers.tile([NP, PFS], BF16, name=f"pd{a}")
            qtiles = {e: pers.tile([NP, PFS], BF16, name=f"q{e}") for e in EXPS_PAIR}
            prod = pers.tile([NP, PFS], BF16)
            acc = pers.tile([NP, PFS], BF16)
            acc2 = pers.tile([NP, PFS], BF16)
            npr5 = pers.tile([NP, PFS], BF16)
            npr8 = pers.tile([NP, PFS], BF16)
            npr9 = pers.tile([NP, PFS], BF16)
            npr10 = pers.tile([NP, PFS], BF16)
            MFS = C * 2 * MW                        # m free size 1368
            mt = pers.tile([NP, MFS], BF16)
            wtile = pers.tile([K90, 2 * OC], BF16)

            # ---------------- loads + zero fills ----------------
            # at/bv first on scalar (d2 needs them immediately); xs on sync.
            nc.sync.dma_start(
                out=_v(at[:], 0, [[2, NP], [1, 2]]),
                in_=_v(av[:], 0, [[2, NP], [1, 2]]),
            )
            nc.sync.dma_start(
                out=_v(bvf[:], 0, [[W, NP], [1, W]]),
                in_=_v(bv[:], 0, [[0, NP], [1, W]]),
            )
            for ci in range(C):
                src = _v(xp[ci], 0, [[2 * WP, NP], [WP, SR], [1, WP]])
                dst = _v(xs[:], ci * SR * WP, [[XFS, NP], [WP, SR], [1, WP]])
                nc.sync.dma_start(out=dst, in_=src)

            nc.gpsimd.memset(mt[:], 0.0)

            # zero mpad rows 0,1 and 226,227 (vertical conv padding)
            for base in (0, MW - 2):
                dst = _v(mpad[:], base * MW, [[MW * MW, C], [1, 2 * MW]])
                src = _v(zv[:], 0, [[2 * MW, C], [1, 2 * MW]])
                nc.scalar.dma_start(out=dst, in_=src)

            nc.scalar.dma_start(
                out=_v(wtile[:], 0, [[2 * OC, K90], [1, 2 * OC]]),
                in_=_v(wc[:], 0, [[2 * OC, K90], [1, 2 * OC]]),
            )

            # ---------------- coefficient chain + rowpairs, interleaved ----
            # vector issue order avoids head-blocking its in-order queue on
            # scalar results: d2 -> rowpairs -> isg -> colpairs -> t1/t2/...
            for rh in range(2):
                nc.vector.tensor_scalar(
                    d2[:, rh * W:(rh + 1) * W], bvf[:],
                    at[:, rh:rh + 1], None, AluOpType.add,
                )
            nc.scalar.activation(dist[:], d2[:], AF.Sqrt)
            nc.scalar.activation(sig[:], dist[:], AF.Copy, bias=0.01, scale=0.99)
            nc.scalar.activation(sqv[:], sig[:], AF.Square)

            # rowpairs, full padded width (independent of coeff chain)
            for a in (1, 2, 3):
                i0 = _v(xs[:], (PG - a) * WP,
                        [[XFS, NP], [SR * WP, C], [1, 2 * WP]])
                i1 = _v(xs[:], (PG + a) * WP,
                        [[XFS, NP], [SR * WP, C], [1, 2 * WP]])
                o = _v(rp[a][:], 0, [[RFS, NP], [1, RFS]])
                nc.vector.tensor_tensor(o, i0, i1, AluOpType.add)

            nc.vector.reciprocal_approx_fast(isg[:], sqv[:])
            nc.scalar.activation(u1f[:], isg[:], AF.Exp, scale=-0.5)
            nc.scalar.activation(u4f[:], isg[:], AF.Exp, scale=-2.0)
            nc.scalar.activation(u9f[:], isg[:], AF.Exp, scale=-4.5)
            for e in ALL_E:
                nc.scalar.activation(ub[e][:], isg[:], AF.Exp, scale=-0.5 * e)

            # ---------------- full-width gaussian pipeline ----------------
            def xsv(col_off):
                return _v(xs[:], PG * WP + PG + col_off,
                          [[XFS, NP], [SR * WP, C], [WP, 2], [1, W]])

            def rpv(a, col_off):
                return _v(rp[a][:], PG + col_off,
                          [[RFS, NP], [2 * WP, C], [WP, 2], [1, W]])

            def pv(t):
                return _v(t[:], 0, [[PFS, NP], [1, PFS]])

            def pv4(t):
                return _v(t[:], 0, [[PFS, NP], [2 * W, C], [W, 2], [1, W]])

            def uv(t):
                return _v(t[:], 0, [[CFS, NP], [0, C], [1, CFS]])

            # colpairs
            for (a, b), pt in ptiles.items():
                if b == 0:
                    continue
                if a == 0:
                    i0, i1 = xsv(-b), xsv(+b)
                else:
                    i0, i1 = rpv(a, -b), rpv(a, +b)
                nc.vector.tensor_tensor(pv4(pt), i0, i1, AluOpType.add)

            # normalizer tail on vector (scalar exps already queued above)
            nc.vector.tensor_tensor(t1[:], u1f[:], u4f[:], AluOpType.add)
            nc.vector.tensor_tensor(t2[:], t1[:], u9f[:], AluOpType.add)
            nc.vector.tensor_scalar(
                sfield[:], t2[:], 2.0, 1.0, AluOpType.mult, AluOpType.add
            )
            nc.vector.reciprocal_approx_fast(rsf[:], sfield[:])
            nc.scalar.activation(rb[:], rsf[:], AF.Square)

            # Q pre-adds
            for e, (a, b) in EXPS_PAIR.items():
                if a == 0:
                    nc.vector.tensor_tensor(
                        pv4(qtiles[e]), pv4(ptiles[(a, b)]), rpv(b, 0),
                        AluOpType.add,
                    )
                else:
                    nc.vector.tensor_tensor(
                        pv(qtiles[e]), pv(ptiles[(a, b)]), pv(ptiles[(b, a)]),
                        AluOpType.add,
                    )

            # products into independent tiles, then depth-3 tree reduction
            # (a single serial accumulator chain stalls the in-order DVE queue)
            sto = {1: qtiles[1], 2: ptiles[(1, 1)], 4: qtiles[4], 5: qtiles[5],
                   8: ptiles[(2, 2)], 9: qtiles[9], 10: qtiles[10]}
            prt = {1: prod, 2: acc, 4: acc2, 5: npr5, 8: npr8, 9: npr9,
                   10: npr10}
            for e in ALL_E:
                nc.vector.tensor_tensor(pv(prt[e]), uv(ub[e]), pv(sto[e]),
                                        AluOpType.mult)
            nc.vector.tensor_tensor(pv(qtiles[1]), pv(prt[1]), pv(prt[2]),
                                    AluOpType.add)
            nc.vector.tensor_tensor(pv(qtiles[4]), pv(prt[4]), pv(prt[5]),
                                    AluOpType.add)
            nc.vector.tensor_tensor(pv(qtiles[5]), pv(prt[8]), pv(prt[9]),
                                    AluOpType.add)
            nc.vector.tensor_tensor(pv4(qtiles[9]), xsv(0), pv4(prt[10]),
                                    AluOpType.add)
            nc.vector.tensor_tensor(pv(qtiles[10]), pv(qtiles[1]),
                                    pv(qtiles[4]), AluOpType.add)
            nc.vector.tensor_tensor(pv(ptiles[(1, 1)]), pv(qtiles[5]),
                                    pv(qtiles[9]), AluOpType.add)
            nc.vector.tensor_tensor(pv(prod), pv(qtiles[10]),
                                    pv(ptiles[(1, 1)]), AluOpType.add)

            # m = rb * numer  (into column-padded mt)
            mdst = _v(mt[:], PC, [[MFS, NP], [2 * MW, C], [MW, 2], [1, W]])
            nc.vector.tensor_tensor(mdst, uv(rb), pv(prod), AluOpType.mult)

            # mt -> mpad rows 2..225 (one DMA per channel; outer dim 112)
            for ci in range(C):
                src = _v(mt[:], ci * 2 * MW, [[MFS, NP], [1, 2 * MW]])
                dst = _v(mpad[:], ci * MW * MW + 2 * MW, [[2 * MW, NP], [1, 2 * MW]])
                nc.sync.dma_start(out=dst, in_=src)

            # ---------------- conv: row strips of RS rows ----------------
            FSIR = FSIF
            for g in range(NSTRIP):
                qa = nc.sync if g % 2 == 0 else nc.scalar
                qb = nc.scalar if g % 2 == 0 else nc.sync
                # hop A: mpad -> imrow[18] fat rows, partitions k' = dy'*3+ci
                imrow = imr_pool.tile([C * NDY + 1, FSIR], BF16, name="imrow")
                srcA = _v(mpad[:], g * RS * MW,
                          [[MW, NDY], [MW * MW, C], [1, RUNF + KC - 1]])
                dstA = _v(imrow[:], 0, [[FSIR, C * NDY], [1, RUNF + KC - 1]])
                qa.dma_start(out=dstA, in_=srcA)

                # hop B: imrow -> icf[90] (partitions k'*5+dx). src folds
                # (k', half) into one 36-count outer dim so descriptors spread
                # over all 16 DMA engines; dst keeps partition dim first.
                icf = icf_pool.tile([K90, FSIF], BF16, name="icf")
                for dx in range(KC):
                    srcB = _v(imrow[:], dx, [[FSIR, C * NDY], [1, RUNF]])
                    dstB = _v(icf[:], dx * FSIF, [[KC * FSIF, C * NDY], [1, RUNF]])
                    (qa if dx % 2 == 0 else qb).dma_start(out=dstB, in_=srcB)

                stage = stg_pool.tile([2 * OC, FSST], BF16, name="stage")
                for ch in range(NCH):
                    ps = psum_pool.tile([2 * OC, NMM], F32, name="ps")
                    rhs = _v(icf[:], 4 * ch * MW,
                             [[FSIF, K90], [2 * MW, 2], [1, W]])
                    nc.tensor.matmul(ps[:], wtile[:], rhs, start=True, stop=True)
                    st_slice = stage[:, ch * NMM:(ch + 1) * NMM]
                    if ch % 2 == 0:
                        nc.scalar.copy(st_slice, ps[:])
                    else:
                        nc.vector.tensor_scalar(
                            st_slice, ps[:], 1.0, None, AluOpType.mult
                        )

                for h in range(2):
                    dst = _v(out[:], g * RS * W + h * W,
                             [[H * W, OC], [2 * W, RS // 2], [1, W]])
                    src = _v(stage[:], h * OC * FSST, [[FSST, OC], [1, FSST]])
                    qb.dma_start(out=dst, in_=src)

    return nc


def _get_nc():
    global _CACHED
    if _CACHED is None:
        nc = _build_nc()
        nc.finalize()
        _CACHED = nc
    return _CACHED


def _host_prep(input_data, foa_xy, weight):
    b = input_data.shape[0]
    wcs = np.zeros((K90, 2 * OC), dtype=np.float32)
    for ci in range(C):
        for dyp in range(NDY):
            for dx in range(KC):
                k = (dyp * C + ci) * KC + dx
                if dyp <= 4:
                    wcs[k, :OC] = weight[:, ci, dyp, dx]
                if dyp >= 1:
                    wcs[k, OC:] = weight[:, ci, dyp - 1, dx]
    wcs = wcs.astype(ml_dtypes.bfloat16)
    idx = np.arange(H, dtype=np.float64)
    in_maps = []
    for i in range(b):
        xpad = np.zeros((C, WP, WP), dtype=ml_dtypes.bfloat16)
        xpad[:, PG:PG + H, PG:PG + W] = input_data[i].astype(ml_dtypes.bfloat16)
        fx, fy = float(foa_xy[i, 0]), float(foa_xy[i, 1])
        a_sq = (((idx - fx) / DNORM) ** 2).astype(np.float32)
        b_sq = (((idx - fy) / DNORM) ** 2).astype(np.float32)
        zvz = np.zeros(2 * C * MW, dtype=ml_dtypes.bfloat16)
        in_maps.append({"xp": xpad, "av": a_sq, "bv": b_sq, "wc": wcs, "zv": zvz})
    return in_maps


def kernel(input_data, foa_xy, weight):
    global LAST_RESULTS
    nc = _get_nc()
    in_maps = _host_prep(np.asarray(input_data), np.asarray(foa_xy),
                         np.asarray(weight))
    trace = bool(int(os.environ.get("BASSKERNEL_TRACE", "0")))
    res = run_bass_kernel_spmd(nc, in_maps, core_ids=list(range(8)), trace=trace)
    LAST_RESULTS = res
    outs = [np.asarray(r["out"], dtype=np.float32) for r in res.results]
    return np.stack(outs, axis=0)
